# revision 27
# baseline (speedup 1.0000x reference)
"""YOLOv1 loss kernel for Trainium2, 8-core data-parallel.

Strategy: shard batch (8192) across 8 cores (1024 rows each). Each core
streams its shard in chunks of g*128 rows laid out as
[128 partitions, g, channels*49] in SBUF. Channel-pair arithmetic uses
strided "pair views" (stride 245 = 5 channels) and stride-0 broadcast
APs so each instruction covers 2-4 channels of both IoU boxes at once.

The four engines run a software-pipelined schedule with fixed stage
offsets (stage(chunk j) issued in iteration j+k), so every consumed
value is >= 1 iteration old and engines never stall on fresh data:

  iter i   DMA : chunk i (box channels, then class channels;
                 class-first on the last chunk to shorten the drain)
           Pool: dsq(i-1), dcls(i-1) x2, mask2(i-1), box-ops(i)
           ACT : qw(i-1), sqrt/conf(i), sqcls_a(i-1), e(i-1), abs(i),
                 sqcls_b(i-1), qxy(i)
           DVE : w2+w3(i-1), dxy(i), cls-tail(i-2), w4-front(i-1)

Loss decomposition (per cell, u = [iou1>=iou2], obj = [l4==1]):
  per_cell = 0.5*nc2 + obj*(t4 + 5*u*(dcoor + 0.1*de))
  t4   = 5*coor2 + e2 + 0.5*e1 + cls - 0.5*nc2
  nc2  = p4^2 + p9^2, e_i = (conf_i - iou_i)^2, de = e1-e2,
  dcoor = coor1-coor2.
IoU in cell-scaled coords: overlap_x = max(0, min(3.5*(wa+wg) - |dcx|,
7*min(wa,wg))); inter49 = ovx*ovy, union49 = 49*(wa*ha+wg*hg) - inter49.

Per-partition accumulators (3 per chunk) via scalar_tensor_tensor
accum_out; host sums 8 cores x 128 partitions x nchunk x 3, divides by B.
"""

import sys

import numpy as np

for _p in ("/opt/trn_rl_repo", "/root/.axon_site/_ro/trn_rl_repo"):
    if _p not in sys.path:
        sys.path.insert(0, _p)

import concourse.bass as bass
import concourse.mybir as mybir
from concourse.ap import AP
from concourse.bass_utils import run_bass_kernel_spmd

F32 = mybir.dt.float32
Alu = mybir.AluOpType
Act = mybir.ActivationFunctionType
BF16 = mybir.dt.bfloat16

B_TOTAL = 8192
NCORES = 8
B_CORE = B_TOTAL // NCORES  # 1024
P = 128
K = 49  # cells
C = 30

CHUNKS = (1, 2, 2, 2, 1)  # g per chunk; sum * 128 == B_CORE
EARLY_TAIL = False
NIB = 3  # input buffers

PT_W = 10 * K   # pred channels 0..9
LT_W = 9 * K    # label channels 0..8 (ch9 unused)
CL_W = 20 * K   # channels 10..29


def pview(tile, offset, dims):
    """Strided free-dim view of an SBUF tile, keeping its partition entry."""
    base = tile[:]
    return AP(base.tensor, offset, [list(base.ap[0])] + [list(d) for d in dims])


def fl(ap4):
    """Contiguous [p,g,c,k] -> [p,g,(c k)] (3D for stt/reciprocal/pool)."""
    return ap4.rearrange("p g c k -> p g (c k)")


def build_nc(chunks=CHUNKS):
    n = len(chunks)
    gmax = max(chunks)
    assert sum(chunks) * P == B_CORE
    nc = bass.Bass()
    pred = nc.declare_dram_parameter("pred", [B_CORE, C * K], F32, isOutput=False)
    labels = nc.declare_dram_parameter("labels", [B_CORE, C * K], F32, isOutput=False)
    out = nc.declare_dram_parameter("out", [P, n * 3], F32, isOutput=True)

    from contextlib import ExitStack

    ctx = ExitStack()
    with ctx:
        def sb(name, shape, dt=F32):
            return ctx.enter_context(nc.sbuf_tensor(name, shape, dt))

        def sb2(name, shape, dt=F32):
            return [sb(f"{name}{b}", shape, dt) for b in range(2)]

        def sb3(name, shape, dt=F32):
            return [sb(f"{name}{b}", shape, dt) for b in range(3)]

        # inputs, triple buffered
        pt = sb3("pt", [P, gmax, PT_W])
        lt = sb3("lt", [P, gmax, LT_W])
        pc = sb3("pc", [P, gmax, CL_W])
        lc = sb3("lc", [P, gmax, CL_W])
        # scratch, double buffered (parity = chunk % 2)
        D = sb2("D", [P, gmax, 6, K])       # dxy1 | dxy2 | dxyc2
        AbsT = sb2("Ab", [P, gmax, 4, K])
        S = sb2("S", [P, gmax, 4, K])
        M = sb2("M", [P, gmax, 4, K])
        T = sb2("T", [P, gmax, 4, K])
        OV = sb2("OV", [P, gmax, 4, K])
        OVC = sb2("OVC", [P, gmax, 4, K])
        INT = sb2("INT", [P, gmax, 2, K])
        AP12 = sb2("AP12", [P, gmax, 2, K])
        AG = sb2("AG", [P, gmax, K])
        U1 = sb2("U1", [P, gmax, 2, K])
        UN = sb2("UN", [P, gmax, 2, K])
        RC = sb2("RC", [P, gmax, 2, K])
        IOU = sb2("IOU", [P, gmax, 2, K])
        USE1 = sb2("USE1", [P, gmax, K])
        DCF = sb2("DCF", [P, gmax, 2, K])
        E = sb2("E", [P, gmax, 2, K])
        AB2 = sb2("AB2", [P, gmax, 2, K])
        SP = sb2("SP", [P, gmax, 4, K])
        SL = sb2("SL", [P, gmax, 4, K])
        DSQ = sb2("DSQ", [P, gmax, 4, K])
        Q = sb2("Q", [P, gmax, 2, 4, 50], BF16)   # cell dim padded to 50
        TQ = sb2("TQ", [P, gmax, 2, 100], BF16)
        T10 = sb2("T10", [P, gmax, 10, 50], BF16)
        T5C = sb2("T5C", [P, gmax, 5, 50], BF16)
        TA = sb2("TA", [P, gmax, 2, 50], BF16)
        TBB = sb2("TBB", [P, gmax, 50], BF16)
        CB = sb2("CB", [P, gmax, 2, K])
        NC2 = sb2("NC2", [P, gmax, K])
        CLA = sb2("CLA", [P, gmax, K])
        DE = sb2("DE", [P, gmax, K])
        DCO = sb2("DCO", [P, gmax, K])
        BASE = sb2("BASE", [P, gmax, K])
        T2 = sb2("T2", [P, gmax, K])
        T5 = sb2("T5", [P, gmax, K])
        T4 = sb2("T4", [P, gmax, K])
        SEL = sb2("SEL", [P, gmax, K])
        JA = sb2("JA", [P, gmax, K])
        JB = sb2("JB", [P, gmax, K])
        DCLS = sb2("DCLS", [P, gmax, 20, K])
        # triple buffered: written by Pool/ACT, read 2 iterations later
        OBJ = sb3("OBJ", [P, gmax, K])
        MK2 = sb3("MK2", [P, gmax, K])
        SQC = sb3("SQC", [P, gmax, 20, 50], BF16)
        acc = sb("acc", [P, n * 3])

        sems = {}
        for nm in ("dma_sem", "sV1", "sV3", "sV4", "v_done", "sA1", "sA2",
                   "sA3", "sA4", "sA5", "sA6", "sP1", "sP2", "sPD", "sP3"):
            sems[nm] = ctx.enter_context(nc.semaphore(nm))
        dma_sem = sems["dma_sem"]
        sV1, sV3, sV4, v_done = sems["sV1"], sems["sV3"], sems["sV4"], sems["v_done"]
        sA1, sA2, sA3 = sems["sA1"], sems["sA2"], sems["sA3"]
        sA4, sA5, sA6 = sems["sA4"], sems["sA5"], sems["sA6"]
        sP1, sP2, sPD, sP3 = sems["sP1"], sems["sP2"], sems["sPD"], sems["sP3"]
        block = ctx.enter_context(nc.Block())

        row_of = [0]
        for g in chunks:
            row_of.append(row_of[-1] + g * P)
        # dma_sem counts at which box / cls parts of chunk j are resident
        boxrdy = [64 * j + 32 if j < n - 1 else 64 * (j + 1) for j in range(n)]
        clsrdy = [64 * (j + 1) if j < n - 1 else 64 * j + 32 for j in range(n)]

        def w(eng, s, v):
            if v > 0:
                eng.wait_ge(s, v)

        @block.sync
        def _(sync):
            for j, g in enumerate(chunks):
                b = j % NIB
                if j >= NIB:
                    w(sync, sV3, j - 2)        # V_23(j-3) done with inputs
                    w(sync, sPD, 2 * (j - 2))  # dcls(j-3) done
                    w(sync, sA1, j - 2)        # A_0(j-3) done
                rows = slice(row_of[j], row_of[j + 1])
                box = [
                    (pt[b][:, :g, :], pred, 0, PT_W),
                    (lt[b][:, :g, :], labels, 0, LT_W),
                ]
                cls = [
                    (pc[b][:, :g, :], pred, PT_W, C * K),
                    (lc[b][:, :g, :], labels, PT_W, C * K),
                ]
                parts = box + cls if j < n - 1 else cls + box
                for o, srct, c0, c1 in parts:
                    sync.dma_start(
                        out=o,
                        in_=srct[rows, c0:c1].rearrange("(g p) d -> p g d", p=P),
                    ).then_inc(dma_sem, 16)
            sync.wait_ge(v_done, n)
            sync.dma_start(out=out[:], in_=acc[:]).then_inc(dma_sem, 16)
            sync.wait_ge(dma_sem, 64 * n + 16)

        @block.gpsimd
        def _(gp):
            for i in range(n + 2):
                if 1 <= i <= n:
                    j, s, b = i - 1, (i - 1) % 2, (i - 1) % NIB
                    g = chunks[i - 1]

                    def dsq_j():
                        w(gp, sA1, j + 1)
                        w(gp, sA4, j - 1)  # qw(j-2) done with DSQ[s]
                        gp.tensor_tensor(fl(DSQ[s][:, :g, :, :]),
                                         fl(SP[s][:, :g, :, :]),
                                         fl(SL[s][:, :g, :, :]), Alu.subtract)
                        gp.drain().then_inc(sP2, 1)

                    def dcls_j():
                        w(gp, dma_sem, clsrdy[j])
                        w(gp, sA6, 2 * (j - 1))  # sqcls(j-2) done w/ DCLS[s]
                        for h in range(2):
                            gp.tensor_tensor(
                                fl(DCLS[s][:, :g, 10 * h:10 * (h + 1), :]),
                                pc[b][:, :g, 490 * h:490 * (h + 1)],
                                lc[b][:, :g, 490 * h:490 * (h + 1)],
                                Alu.subtract)
                            gp.drain().then_inc(sPD, 1)

                    # last chunk loads class channels first: diff them first
                    if j == n - 1:
                        dcls_j()
                        dsq_j()
                    else:
                        dsq_j()
                        dcls_j()
                    # mask2(j) = obj * use1
                    w(gp, sV3, j + 1)
                    w(gp, v_done, j - 2)  # V_T(j-3) done with MK2[j%3]
                    gp.tensor_tensor(MK2[j % NIB][:, :g, :],
                                     OBJ[j % NIB][:, :g, :],
                                     USE1[s][:, :g, :], Alu.mult)
                    gp.drain().then_inc(sP3, 1)
                if i < n:
                    j, s, b = i, i % 2, i % NIB
                    g = chunks[i]
                    w(gp, dma_sem, boxrdy[j])
                    w(gp, sV3, j - 1)     # V_23(j-2) done with S/D/AP12/AG
                    w(gp, sA3, j - 1)     # qxy(j-2) done with D[4:6]
                    w(gp, v_done, j - 1)  # V_T(j-3) done with OBJ[j%3]
                    p, l = pt[b], lt[b]
                    pch = p[:, :g, :].rearrange("p g (c k) -> p g c k", c=10)
                    lch = l[:, :g, :].rearrange("p g (c k) -> p g c k", c=9)
                    # S = [pw1+lw, ph1+lh, pw2+lw, ph2+lh]
                    gp.tensor_tensor(fl(S[s][:, :g, 0:2, :]),
                                     fl(pch[:, :, 2:4, :]),
                                     fl(lch[:, :, 2:4, :]), Alu.add)
                    gp.tensor_tensor(fl(S[s][:, :g, 2:4, :]),
                                     fl(pch[:, :, 7:9, :]),
                                     fl(lch[:, :, 2:4, :]), Alu.add)
                    # dxyc2 = p(5,6) - l(5,6)
                    gp.tensor_tensor(fl(D[s][:, :g, 4:6, :]),
                                     fl(pch[:, :, 5:7, :]),
                                     fl(lch[:, :, 5:7, :]), Alu.subtract)
                    # objm = (l4 == 1)
                    gp.tensor_scalar(OBJ[j % NIB][:, :g, :], lch[:, :, 4, :],
                                     1.0, None, Alu.is_equal)
                    # areas: ap12 = [pw1*ph1, pw2*ph2]; ag = lw*lh
                    gp.tensor_tensor(AP12[s][:, :g, :, :], pch[:, :, 2:8:5, :],
                                     pch[:, :, 3:9:5, :], Alu.mult)
                    gp.tensor_tensor(AG[s][:, :g, :], lch[:, :, 2, :],
                                     lch[:, :, 3, :], Alu.mult)
                    gp.drain().then_inc(sP1, 1)

        @block.scalar
        def _(act):
            for i in range(n + 2):
                if 1 <= i <= n:
                    j, s, g = i - 1, (i - 1) % 2, chunks[i - 1]
                    # qw(j): sqrt-diff squares into Q slots [pair][2:4]
                    w(act, sP2, j + 1)
                    w(act, sV4, j - 1)  # V_4(j-2) done with Q[s]
                    qw_in = DSQ[s][:, :g, :, :].rearrange(
                        "p g (pr c) k -> p g pr c k", pr=2)
                    qw_o = pview(Q[s], 100,
                                 [[400, g], [200, 2], [50, 2], [1, K]])
                    act.activation(qw_o, qw_in, Act.Square)
                    act.drain().then_inc(sA4, 1)
                if i < n:
                    j, s, g, b = i, i % 2, chunks[i], i % NIB
                    # sqrt of w/h channels + conf squares of chunk j
                    w(act, dma_sem, boxrdy[j])
                    w(act, sP2, j - 1)  # dsq(j-2) done with SP/SL[s]
                    w(act, sV4, j - 1)  # V_4(j-2) done with AB2[s]
                    p, l = pt[b], lt[b]
                    pch = p[:, :g, :].rearrange("p g (c k) -> p g c k", c=10)
                    p_wh = pview(p, 2 * K, [[PT_W, g], [5 * K, 2], [K, 2], [1, K]])
                    l_wh = pview(l, 2 * K, [[LT_W, g], [5 * K, 2], [K, 2], [1, K]])
                    sp_o = SP[s][:, :g, :, :].rearrange(
                        "p g (pr c) k -> p g pr c k", pr=2)
                    sl_o = SL[s][:, :g, :, :].rearrange(
                        "p g (pr c) k -> p g pr c k", pr=2)
                    act.activation(sp_o, p_wh, Act.Sqrt)
                    act.activation(sl_o, l_wh, Act.Sqrt)
                    act.activation(AB2[s][:, :g, :, :], pch[:, :, 4:10:5, :],
                                   Act.Square)
                    act.drain().then_inc(sA1, 1)
                if 1 <= i <= n:
                    j, s, g = i - 1, (i - 1) % 2, chunks[i - 1]
                    # e(j) = (conf - iou)^2  (consumed this iteration)
                    w(act, sV3, j + 1)
                    w(act, sV4, j - 1)  # V_4(j-2) done with E[s]
                    act.activation(E[s][:, :g, :, :], DCF[s][:, :g, :, :],
                                   Act.Square)
                    act.drain().then_inc(sA5, 1)
                    # class squares half a of chunk j
                    w(act, sPD, 2 * j + 1)
                    w(act, v_done, j - 1)  # V_T(j-2) done with SQC[j%3]
                    act.activation(
                        SQC[j % NIB][:, :g, 0:10, 0:49],
                        DCLS[s][:, :g, 0:10, :], Act.Square)
                    act.drain().then_inc(sA6, 1)
                if i < n:
                    j, s, g = i, i % 2, chunks[i]
                    # |d| of the four IoU center diffs of chunk j
                    w(act, sV1, j + 1)
                    w(act, sV3, j - 1)  # V_23(j-2) done with AbsT[s]
                    act.activation(AbsT[s][:, :g, :, :], D[s][:, :g, 0:4, :],
                                   Act.Abs)
                    act.drain().then_inc(sA2, 1)
                if 1 <= i <= n:
                    j, s, g = i - 1, (i - 1) % 2, chunks[i - 1]
                    # class squares half b of chunk j
                    w(act, sPD, 2 * j + 2)
                    act.activation(
                        SQC[j % NIB][:, :g, 10:20, 0:49],
                        DCLS[s][:, :g, 10:20, :], Act.Square)
                    act.drain().then_inc(sA6, 1)
                if i < n:
                    j, s, g = i, i % 2, chunks[i]
                    # qxy(j): xy-diff squares into Q slots [pair][0:2]
                    w(act, sP1, j + 1)
                    w(act, sV4, j - 1)  # V_4(j-2) done with Q[s]
                    qxy_in = pview(D[s], 0,
                                   [[6 * K, g], [4 * K, 2], [K, 2], [1, K]])
                    qxy_o = pview(Q[s], 0,
                                  [[400, g], [200, 2], [50, 2], [1, K]])
                    act.activation(qxy_o, qxy_in, Act.Square)
                    act.drain().then_inc(sA3, 1)

        @block.vector
        def _(v):
            stt = v.scalar_tensor_tensor
            tt = v.tensor_tensor
            for i in range(n + 2):
                if 1 <= i <= n:
                    # w2+w3 of chunk j = i-1
                    j, s, g, b = i - 1, (i - 1) % 2, chunks[i - 1], (i - 1) % NIB
                    w(v, sA2, j + 1)
                    w(v, sP1, j + 1)
                    w(v, sP3, j - 1)  # mask2(j-2) done with USE1[s]
                    w(v, sA5, j - 1)  # e(j-2) done with DCF[s]
                    p, l = pt[b], lt[b]
                    pch = p[:, :g, :].rearrange("p g (c k) -> p g c k", c=10)
                    lch = l[:, :g, :].rearrange("p g (c k) -> p g c k", c=9)
                    p_wh = pview(p, 2 * K, [[PT_W, g], [5 * K, 2], [K, 2], [1, K]])
                    l_wh_b = lch[:, :, 2:4, :].unsqueeze(2).broadcast_to(
                        (P, g, 2, 2, K))
                    m_o = M[s][:, :g, :, :].rearrange(
                        "p g (pr c) k -> p g pr c k", pr=2)
                    tt(m_o, p_wh, l_wh_b, Alu.min)
                    stt(fl(T[s][:, :g, :, :]), fl(S[s][:, :g, :, :]), 3.5,
                        fl(AbsT[s][:, :g, :, :]), Alu.mult, Alu.subtract)
                    stt(fl(OV[s][:, :g, :, :]), fl(M[s][:, :g, :, :]), 7.0,
                        fl(T[s][:, :g, :, :]), Alu.mult, Alu.min)
                    v.tensor_scalar(fl(OVC[s][:, :g, :, :]),
                                    fl(OV[s][:, :g, :, :]), 0.0, None, Alu.max)
                    ovc = OVC[s][:, :g, :, :].rearrange(
                        "p g (pr c) k -> p g pr c k", pr=2)
                    tt(INT[s][:, :g, :, :], ovc[:, :, :, 0, :],
                       ovc[:, :, :, 1, :], Alu.mult)
                    stt(fl(U1[s][:, :g, :, :]), fl(AP12[s][:, :g, :, :]), 49.0,
                        fl(INT[s][:, :g, :, :]), Alu.mult, Alu.subtract)
                    for pr in range(2):
                        stt(UN[s][:, :g, pr, :], AG[s][:, :g, :], 49.0,
                            U1[s][:, :g, pr, :], Alu.mult, Alu.add)
                    v.reciprocal(fl(RC[s][:, :g, :, :]), fl(UN[s][:, :g, :, :]))
                    tt(IOU[s][:, :g, :, :], INT[s][:, :g, :, :],
                       RC[s][:, :g, :, :], Alu.mult)
                    iouv = IOU[s][:, :g, :, :]
                    tt(USE1[s][:, :g, :], iouv[:, :, 0, :], iouv[:, :, 1, :],
                       Alu.is_ge)
                    tt(DCF[s][:, :g, :, :], pch[:, :, 4:10:5, :], iouv,
                       Alu.subtract)
                    v.drain().then_inc(sV3, 1)
                if i < n:
                    # w1 (dxy) of chunk j = i
                    j, s, g, b = i, i % 2, chunks[i], i % NIB
                    w(v, dma_sem, boxrdy[j])
                    w(v, sA3, j - 1)  # qxy(j-2) done with D[s]
                    p, l = pt[b], lt[b]
                    lch = l[:, :g, :].rearrange("p g (c k) -> p g c k", c=9)
                    p_xy = pview(p, 0, [[PT_W, g], [5 * K, 2], [K, 2], [1, K]])
                    l_xy = lch[:, :, 0:2, :].unsqueeze(2).broadcast_to(
                        (P, g, 2, 2, K))
                    d_o = D[s][:, :g, 0:4, :].rearrange(
                        "p g (pr c) k -> p g pr c k", pr=2)
                    tt(d_o, p_xy, l_xy, Alu.subtract).then_inc(sV1, 1)
                if 2 <= i <= n + 1 and not (i - 2 >= n - 1 and EARLY_TAIL):
                    # class tail of chunk j = i-2: bf16 add-tree 20 -> 1
                    j, s, g = i - 2, (i - 2) % 2, chunks[i - 2]
                    sq = SQC[j % NIB]
                    w(v, sA6, 2 * j + 2)
                    tt(T10[s][:, :g, :, :].rearrange("p g c k -> p g (c k)"),
                       sq[:, :g, 0:10, :].rearrange("p g c k -> p g (c k)"),
                       sq[:, :g, 10:20, :].rearrange("p g c k -> p g (c k)"),
                       Alu.add)
                    tt(T5C[s][:, :g, :, :].rearrange("p g c k -> p g (c k)"),
                       T10[s][:, :g, 0:5, :].rearrange("p g c k -> p g (c k)"),
                       T10[s][:, :g, 5:10, :].rearrange("p g c k -> p g (c k)"),
                       Alu.add)
                    tt(TA[s][:, :g, :, :].rearrange("p g c k -> p g (c k)"),
                       T5C[s][:, :g, 0:2, :].rearrange("p g c k -> p g (c k)"),
                       T5C[s][:, :g, 2:4, :].rearrange("p g c k -> p g (c k)"),
                       Alu.add)
                    tt(TBB[s][:, :g, :], TA[s][:, :g, 0, :], TA[s][:, :g, 1, :],
                       Alu.add)
                    tt(CLA[s][:, :g, :], TBB[s][:, :g, 0:49],
                       T5C[s][:, :g, 4, 0:49], Alu.add)
                    tt(T5[s][:, :g, :], T2[s][:, :g, :], CLA[s][:, :g, :],
                       Alu.add)
                    stt(T4[s][:, :g, :], NC2[s][:, :g, :], -0.5,
                        T5[s][:, :g, :], Alu.mult, Alu.add)
                    stt(JA[s][:, :g, :], OBJ[j % NIB][:, :g, :], 1.0,
                        T4[s][:, :g, :], Alu.mult, Alu.mult,
                        accum_out=acc[:, 3 * j + 1:3 * j + 2])
                    w(v, sP3, j + 1)
                    stt(JB[s][:, :g, :], MK2[j % NIB][:, :g, :], 5.0,
                        SEL[s][:, :g, :], Alu.mult, Alu.mult,
                        accum_out=acc[:, 3 * j + 2:3 * j + 3])
                    v.drain().then_inc(v_done, 1)
                if 1 <= i <= n:
                    # w4 front of chunk j = i-1
                    j, s, g = i - 1, (i - 1) % 2, chunks[i - 1]
                    w(v, sA4, j + 1)
                    w(v, sA3, j + 1)
                    qv = Q[s][:, :g, :, :, :]
                    tt(TQ[s][:, :g, :, :],
                       qv[:, :, :, 0:2, :].rearrange("p g pr c k -> p g pr (c k)"),
                       qv[:, :, :, 2:4, :].rearrange("p g pr c k -> p g pr (c k)"),
                       Alu.add)
                    tt(CB[s][:, :g, :, :], TQ[s][:, :g, :, 0:49],
                       TQ[s][:, :g, :, 50:99], Alu.add)
                    w(v, sA1, j + 1)
                    ab2 = AB2[s][:, :g, :, :]
                    stt(NC2[s][:, :g, :], ab2[:, :, 0, :], 1.0, ab2[:, :, 1, :],
                        Alu.mult, Alu.add,
                        accum_out=acc[:, 3 * j + 0:3 * j + 1])
                    w(v, sA5, j + 1)
                    ev = E[s][:, :g, :, :]
                    cbv = CB[s][:, :g, :, :]
                    tt(DE[s][:, :g, :], ev[:, :, 0, :], ev[:, :, 1, :],
                       Alu.subtract)
                    tt(DCO[s][:, :g, :], cbv[:, :, 0, :], cbv[:, :, 1, :],
                       Alu.subtract)
                    stt(BASE[s][:, :g, :], cbv[:, :, 1, :], 5.0, ev[:, :, 1, :],
                        Alu.mult, Alu.add)
                    stt(T2[s][:, :g, :], ev[:, :, 0, :], 0.5, BASE[s][:, :g, :],
                        Alu.mult, Alu.add)
                    stt(SEL[s][:, :g, :], DE[s][:, :g, :], 0.1, DCO[s][:, :g, :],
                        Alu.mult, Alu.add)
                    v.drain().then_inc(sV4, 1)
                    if j >= n - 1 and EARLY_TAIL:
                        # drain: run this chunk's class tail immediately
                        sq = SQC[j % NIB]
                        w(v, sA6, 2 * j + 2)
                        tt(T10[s][:, :g, :, :].rearrange("p g c k -> p g (c k)"),
                           sq[:, :g, 0:10, :].rearrange("p g c k -> p g (c k)"),
                           sq[:, :g, 10:20, :].rearrange("p g c k -> p g (c k)"),
                           Alu.add)
                        tt(T5C[s][:, :g, :, :].rearrange("p g c k -> p g (c k)"),
                           T10[s][:, :g, 0:5, :].rearrange("p g c k -> p g (c k)"),
                           T10[s][:, :g, 5:10, :].rearrange("p g c k -> p g (c k)"),
                           Alu.add)
                        tt(TA[s][:, :g, :, :].rearrange("p g c k -> p g (c k)"),
                           T5C[s][:, :g, 0:2, :].rearrange("p g c k -> p g (c k)"),
                           T5C[s][:, :g, 2:4, :].rearrange("p g c k -> p g (c k)"),
                           Alu.add)
                        tt(TBB[s][:, :g, :], TA[s][:, :g, 0, :],
                           TA[s][:, :g, 1, :], Alu.add)
                        tt(CLA[s][:, :g, :], TBB[s][:, :g, 0:49],
                           T5C[s][:, :g, 4, 0:49], Alu.add)
                        tt(T5[s][:, :g, :], T2[s][:, :g, :], CLA[s][:, :g, :],
                           Alu.add)
                        stt(T4[s][:, :g, :], NC2[s][:, :g, :], -0.5,
                            T5[s][:, :g, :], Alu.mult, Alu.add)
                        stt(JA[s][:, :g, :], OBJ[j % NIB][:, :g, :], 1.0,
                            T4[s][:, :g, :], Alu.mult, Alu.mult,
                            accum_out=acc[:, 3 * j + 1:3 * j + 2])
                        w(v, sP3, j + 1)
                        stt(JB[s][:, :g, :], MK2[j % NIB][:, :g, :], 5.0,
                            SEL[s][:, :g, :], Alu.mult, Alu.mult,
                            accum_out=acc[:, 3 * j + 2:3 * j + 3])
                        v.drain().then_inc(v_done, 1)

    return nc


_NC_CACHE = {}


def _get_nc():
    if "nc" not in _NC_CACHE:
        _NC_CACHE["nc"] = build_nc()
    return _NC_CACHE["nc"]


def run_device(pred, labels, trace=False):
    nc = _get_nc()
    pred = np.ascontiguousarray(pred, dtype=np.float32).reshape(B_TOTAL, C * K)
    labels = np.ascontiguousarray(labels, dtype=np.float32).reshape(B_TOTAL, C * K)
    in_maps = []
    for c in range(NCORES):
        rows = slice(c * B_CORE, (c + 1) * B_CORE)
        in_maps.append({"pred": pred[rows], "labels": labels[rows]})
    res = run_bass_kernel_spmd(nc, in_maps, list(range(NCORES)), trace=trace)
    total = 0.0
    for c in range(NCORES):
        arr = res.results[c]["out"].astype(np.float64).reshape(P, len(CHUNKS), 3)
        total += 0.5 * arr[:, :, 0].sum() + arr[:, :, 1].sum() + arr[:, :, 2].sum()
    loss = np.float32(total / B_TOTAL)
    return loss, res


def kernel(pred, labels):
    loss, _ = run_device(pred, labels, trace=False)
    return np.array(loss, dtype=np.float32)


if __name__ == "__main__":
    rng = np.random.default_rng(0)
    p = rng.random((B_TOTAL, C, 7, 7), dtype=np.float32)
    l = rng.random((B_TOTAL, C, 7, 7), dtype=np.float32)
    l[:, 4] = (rng.random((B_TOTAL, 7, 7)) < 0.3).astype(np.float32)
    print(kernel(p, l))
